# revision 27
# baseline (speedup 1.0000x reference)
"""Trainium2 Bass kernel for nn_B_MLUTNet (binarized MLP + BatchNorm + log_softmax).

V2 strategy (vs V1 baseline at 1.163 ms):
  - Data-parallel over batch: 8192 rows -> 8 cores x 1024 rows.
  - Feature-major [h, b] layout through layers 1-4; operand swap on the head.
  - All matmuls fp8 DoubleRow (sign values exact in fp8, fp32 PSUM).
  - Host pre-signs x into DR-tiled fp8 (kills on-chip sign pass + 4x less DMA).
  - Stats slimmed: S1 on the Pool engine (reduce of the f16 m tile), S2 via one
    DVE tensor_tensor_reduce per psum block (kills the Square acts + reduces).
  - Phase overlap: normalize passes are emission-interleaved into the next
    matmul phase so PE never idles: s1p into L2, sup into L3, s4p into L4's
    tail, svp into the head-B matmuls; per-bt softmax into the head.
  - AR3/AR4 split in halves so norm 3/4 can start mid-L4.
  - m1..m4 spill to DRAM f16 (exact ints) and stream back under matmul phases.
  - Head weights streamed per-kc from DRAM (kc-outer loops) to keep SBUF small.
"""

from contextlib import ExitStack

import numpy as np
import ml_dtypes

from concourse import bass, bacc, tile, mybir
from concourse.bass_utils import run_bass_kernel_spmd

FP8 = mybir.dt.float8e4
F16 = mybir.dt.float16
F32 = mybir.dt.float32
AF = mybir.ActivationFunctionType
ALU = mybir.AluOpType
DR = mybir.MatmulPerfMode.DoubleRow
AX = mybir.AxisListType

D_IN, H, D_OUT, B = 1024, 4096, 1000, 8192
EPS = 1e-5
N_CORES = 8
HT = H // 128          # 32 h-tiles
KT_X = D_IN // 128     # 8 k-tiles per input half
P = 128

f8np = ml_dtypes.float8_e4m3

SPLIT_AR = False  # half-layer AllReduces for L3/L4 (6 collectives vs 4)
OVERLAP = True    # interleave normalize passes into matmul loops


class _Overlay:
    """Queue of closures emitted interleaved into a matmul loop."""

    def __init__(self):
        self.items = []

    def add(self, fn):
        self.items.append(fn)

    def drain(self, k=None):
        n = len(self.items) if k is None else min(k, len(self.items))
        for _ in range(n):
            self.items.pop(0)()

    def __len__(self):
        return len(self.items)


class _Builder:
    def __init__(self, nc, tc, b_loc, n_cores):
        self.nc = nc
        self.tc = tc
        self.b_loc = b_loc
        self.n_cores = n_cores
        self.nblk = 512
        self.nb = b_loc // self.nblk
        self.bt_tiles = b_loc // 128
        self.b_glob = b_loc * n_cores
        self.groups = [list(range(n_cores))]

    def declare_io(self):
        nc = self.nc
        b_loc = self.b_loc
        # pre-signed x, DR tiled: [8, P, 2, b] (cg 0-3 half1, 4-7 half2)
        self.xs = nc.declare_dram_parameter("xs", [2 * KT_X // 2, P, 2, b_loc], FP8, isOutput=False)
        self.w1t = nc.declare_dram_parameter("w1t", [HT, P, KT_X, P], FP8, isOutput=False)
        self.w2t = nc.declare_dram_parameter("w2t", [HT, P, KT_X, P], FP8, isOutput=False)
        self.w3t = nc.declare_dram_parameter("w3t", [HT, P, HT, P], FP8, isOutput=False)
        self.w4t = nc.declare_dram_parameter("w4t", [HT, P, HT, P], FP8, isOutput=False)
        self.w56 = nc.declare_dram_parameter("w56", [P, HT, 2 * D_OUT], FP8, isOutput=False)
        self.bias56 = nc.declare_dram_parameter("bias56", [P, 2 * D_OUT], F32, isOutput=False)
        self.gb = nc.declare_dram_parameter("gb", [P, 4 * 2 * HT], F32, isOutput=False)
        self.outp = nc.declare_dram_parameter("out", [b_loc, 2 * D_OUT], F32, isOutput=True)

    def global_pools(self, st):
        tc, nc = self.tc, self.nc
        self.dram = st.enter_context(tc.tile_pool(name="dram", bufs=1, space="DRAM"))
        self.psum = st.enter_context(tc.tile_pool(name="psum", bufs=8, space="PSUM"))
        self.statsp = st.enter_context(tc.tile_pool(name="stats", bufs=1))
        self.small = st.enter_context(tc.tile_pool(name="small", bufs=6))
        self.sqp = st.enter_context(tc.tile_pool(name="sqp", bufs=3))

        # DRAM spill for m1..m4 (f16 exact ints)
        self.md = [self.dram.tile([H, self.b_loc], F16, tag=f"m{l}d", name=f"m{l}d")
                   for l in range(4)]
        # AllReduce buffers: full for layers 0/1, halves for 2/3
        self.ar_in = {}
        self.ar_out = {}
        for li in (0, 1, 2, 3):
            self.ar_in[li] = self.dram.tile([P, 2 * HT], F32, tag=f"ari{li}", name=f"ari{li}")
            self.ar_out[li] = self.dram.tile([P, 2 * HT], F32, tag=f"aro{li}", name=f"aro{li}")
        if SPLIT_AR:
            for li in (2, 3):
                for h in (0, 1):
                    self.ar_in[(li, h)] = self.dram.tile([P, HT], F32, tag=f"ari{li}_{h}", name=f"ari{li}_{h}")
                    self.ar_out[(li, h)] = self.dram.tile([P, HT], F32, tag=f"aro{li}_{h}", name=f"aro{li}_{h}")

        sp = self.statsp
        self.s1stat = [sp.tile([P, HT], F32, tag=f"s1s{l}", name=f"s1s{l}") for l in range(4)]
        self.s2stat = [sp.tile([P, HT], F32, tag=f"s2s{l}", name=f"s2s{l}") for l in range(4)]
        self.bnagg = [sp.tile([P, HT, 2], F32, tag=f"bna{l}", name=f"bna{l}") for l in range(4)]
        self.alpha = [sp.tile([P, HT], F32, tag=f"al{l}", name=f"al{l}") for l in range(4)]
        self.beta = [sp.tile([P, HT], F32, tag=f"be{l}", name=f"be{l}") for l in range(4)]
        self.gbsb = sp.tile([P, 4 * 2 * HT], F32, tag="gbsb", name="gbsb")
        nc.sync.dma_start(out=self.gbsb[:, :], in_=self.gb[:, :])

    # ---- matmul layer with stats + overlay interleave ----

    def layer(self, li, wt, acts, kt, mpool, mtag, wpool, wtag, wshape,
              overlay, ov_start, ar_cb=None):
        """One of layers 1-4; spills m tiles to self.md[li]."""
        nc = self.nc
        n_m = HT
        for m in range(n_m):
            wsb = wpool.tile(wshape, FP8, tag=wtag, name=f"w{li}_{m}")
            nc.sync.dma_start(out=wsb[:, :, :], in_=wt[m])
            psA = self.psum.tile([P, self.nblk], F32, tag="ps", name=f"psA{li}_{m}")
            psB = self.psum.tile([P, self.nblk], F32, tag="ps", name=f"psB{li}_{m}")
            nchunk = kt // 2
            for kc in range(nchunk):
                nc.tensor.matmul(
                    psA[:, :], lhsT=wsb[:, 2 * kc:2 * kc + 2, :],
                    rhs=acts[kc][:, :, 0:self.nblk],
                    start=(kc == 0), stop=(kc == nchunk - 1), perf_mode=DR,
                )
                nc.tensor.matmul(
                    psB[:, :], lhsT=wsb[:, 2 * kc:2 * kc + 2, :],
                    rhs=acts[kc][:, :, self.nblk:2 * self.nblk],
                    start=(kc == 0), stop=(kc == nchunk - 1), perf_mode=DR,
                )
            mt = mpool.tile([P, self.b_loc], F16, tag=mtag, name=f"mt{li}_{m}")
            nc.scalar.activation(mt[:, 0:self.nblk], psA[:, :], AF.Copy)
            nc.scalar.activation(mt[:, self.nblk:], psB[:, :], AF.Copy)
            # stats via bn_stats/bn_aggr (standard BIR): per-core mean/var of
            # this m-tile land in bnagg[li][:, m, :]; derived to (mean, E[x^2])
            # per AR slice in issue_ar.
            bnst = self.small.tile([P, 12], F32, tag="bnst", name=f"bnst{li}_{m}")
            nc.vector.bn_stats(bnst[:, 0:6], psA[:, :])
            nc.vector.bn_stats(bnst[:, 6:12], psB[:, :])
            nc.vector.bn_aggr(self.bnagg[li][:, m, :], bnst[:, :])
            nc.sync.dma_start(out=self.md[li][m * P:(m + 1) * P, :], in_=mt[:, :])
            if ar_cb is not None:
                ar_cb(m)
            if OVERLAP and m >= ov_start and len(overlay):
                remain_m = n_m - m
                k = max(1, -(-len(overlay) // remain_m))
                overlay.drain(k)

    # ---- AllReduce + alpha/beta ----

    def issue_ar(self, li, cols, key):
        nc = self.nc
        c0, c1 = cols
        w = c1 - c0
        kstr = f"{li}_{c0}"
        # derive per-core (mean, E[x^2]) for this slice from bnagg
        nc.vector.tensor_copy(
            out=self.s1stat[li][:, c0:c1], in_=self.bnagg[li][:, c0:c1, 0:1])
        msq0 = self.statsp.tile([P, w], F32, tag=f"mq{kstr}", name=f"mq{kstr}")
        nc.vector.tensor_tensor(
            out=msq0[:, :], in0=self.s1stat[li][:, c0:c1],
            in1=self.s1stat[li][:, c0:c1], op=ALU.mult)
        nc.vector.tensor_tensor(
            out=self.s2stat[li][:, c0:c1], in0=self.bnagg[li][:, c0:c1, 1:2],
            in1=msq0[:, :], op=ALU.add)
        ari, aro = self.ar_in[key], self.ar_out[key]
        nc.sync.dma_start(out=ari[:, 0:w], in_=self.s1stat[li][:, c0:c1])
        nc.sync.dma_start(out=ari[:, w:2 * w], in_=self.s2stat[li][:, c0:c1])
        nc.gpsimd.collective_compute(
            "AllReduce", ALU.add, replica_groups=self.groups,
            ins=[ari.opt()], outs=[aro.opt()],
        )
        sp = self.statsp
        g12 = sp.tile([P, 2 * w], F32, tag=f"g12_{kstr}", name=f"g12_{kstr}")
        nc.sync.dma_start(out=g12[:, :], in_=aro[:, 0:2 * w])
        mean = sp.tile([P, w], F32, tag=f"mean{kstr}", name=f"mean{kstr}")
        var = sp.tile([P, w], F32, tag=f"var{kstr}", name=f"var{kstr}")
        # g12 holds (sum of per-core means, sum of per-core E[x^2])
        nc.vector.tensor_scalar_mul(mean[:, :], g12[:, 0:w], 1.0 / self.n_cores)
        nc.vector.tensor_scalar_mul(var[:, :], g12[:, w:2 * w], 1.0 / self.n_cores)
        msq = sp.tile([P, w], F32, tag=f"msq{kstr}", name=f"msq{kstr}")
        nc.vector.tensor_tensor(out=msq[:, :], in0=mean[:, :], in1=mean[:, :], op=ALU.mult)
        nc.vector.tensor_tensor(out=var[:, :], in0=var[:, :], in1=msq[:, :], op=ALU.subtract)
        nc.vector.tensor_scalar_add(var[:, :], var[:, :], EPS)
        sd = sp.tile([P, w], F32, tag=f"sd{kstr}", name=f"sd{kstr}")
        nc.scalar.activation(sd[:, :], var[:, :], AF.Sqrt, bias=0.0, scale=1.0)
        rinv = sp.tile([P, w], F32, tag=f"rinv{kstr}", name=f"rinv{kstr}")
        nc.vector.reciprocal(rinv[:, :], sd[:, :])
        gofs = li * 2 * HT
        nc.vector.tensor_tensor(
            out=self.alpha[li][:, c0:c1], in0=self.gbsb[:, gofs + c0:gofs + c1],
            in1=rinv[:, :], op=ALU.mult,
        )
        nc.vector.tensor_tensor(out=msq[:, :], in0=mean[:, :], in1=self.alpha[li][:, c0:c1], op=ALU.mult)
        nc.vector.tensor_tensor(
            out=self.beta[li][:, c0:c1],
            in0=self.gbsb[:, gofs + HT + c0:gofs + HT + c1],
            in1=msq[:, :], op=ALU.subtract,
        )


def build_program(b_loc: int, n_cores: int = N_CORES, finalize: bool = True):
    nc = bacc.Bacc()
    with tile.TileContext(nc) as tc:
        bld = _Builder(nc, tc, b_loc, n_cores)
        bld.declare_io()
        NB = bld.nblk
        with ExitStack() as top:
            bld.global_pools(top)
            # flat program-long pools; reuse is via tag rotation (WAR deps)
            zp = top.enter_context(tc.tile_pool(name="zp", bufs=2))
            xp = top.enter_context(tc.tile_pool(name="xp", bufs=8))
            wp = top.enter_context(tc.tile_pool(name="wp", bufs=3))
            mtp = top.enter_context(tc.tile_pool(name="mtp", bufs=4))
            sg = top.enter_context(tc.tile_pool(name="sg", bufs=32))
            ring = top.enter_context(tc.tile_pool(name="ring", bufs=6))
            yp = top.enter_context(tc.tile_pool(name="yp", bufs=2))
            y6p = top.enter_context(tc.tile_pool(name="y6p", bufs=8))
            bsp = top.enter_context(tc.tile_pool(name="bsp", bufs=1))

            def sign_step(li, t, src_ap, dst_pair):
                c, i = t // 2, t % 2
                nc.scalar.activation(
                    dst_pair[c][:, i, :], src_ap, AF.Sign,
                    bias=bld.beta[li][:, t:t + 1], scale=bld.alpha[li][:, t:t + 1],
                )

            def pair_parts(la, lb, t, srca_ap, srcb_ap, dst_pair):
                """(part1, part2): clip-sum compute, then (later) the sign."""
                holder = {}

                def part1():
                    za = zp.tile([P, b_loc], F32, tag="za", name=f"za{la}_{t}")
                    nc.scalar.activation(za[:, :], srca_ap(), AF.Identity,
                                         bias=bld.beta[la][:, t:t + 1],
                                         scale=bld.alpha[la][:, t:t + 1])
                    nc.vector.tensor_scalar(out=za[:, :], in0=za[:, :], scalar1=1.0,
                                            scalar2=-1.0, op0=ALU.min, op1=ALU.max)
                    zb = zp.tile([P, b_loc], F32, tag="zb", name=f"zb{lb}_{t}")
                    nc.scalar.activation(zb[:, :], srcb_ap(), AF.Identity,
                                         bias=bld.beta[lb][:, t:t + 1],
                                         scale=bld.alpha[lb][:, t:t + 1])
                    nc.vector.tensor_scalar(out=zb[:, :], in0=zb[:, :], scalar1=1.0,
                                            scalar2=-1.0, op0=ALU.min, op1=ALU.max)
                    nc.vector.tensor_tensor(out=za[:, :], in0=za[:, :], in1=zb[:, :], op=ALU.add)
                    holder["za"] = za

                def part2():
                    c, i = t // 2, t % 2
                    nc.scalar.activation(dst_pair[c][:, i, :], holder["za"][:, :], AF.Sign)

                return part1, part2

            def rtile(nm, src_dram, t):
                r = ring.tile([P, b_loc], F16, tag="r", name=f"{nm}_{t}")
                nc.sync.dma_start(out=r[:, :], in_=src_dram[t * P:(t + 1) * P, :])
                return r[:, :]

            # ---------------- phase A: L1 + L2 ----------------
            sx1 = [xp.tile([P, 2, b_loc], FP8, tag="x", name=f"sx{c}") for c in range(4)]
            sx2 = [xp.tile([P, 2, b_loc], FP8, tag="x", name=f"sy{c}") for c in range(4)]
            for c in range(4):
                nc.sync.dma_start(out=sx1[c][:, :, :], in_=bld.xs[c])
            for c in range(4):
                nc.sync.dma_start(out=sx2[c][:, :, :], in_=bld.xs[4 + c])

            ov_none = _Overlay()
            bld.layer(0, bld.w1t, sx1, KT_X, mtp, "mt", wp, "w",
                      [P, KT_X, P], ov_none, HT)
            bld.issue_ar(0, (0, HT), 0)

            # sign sets: sg slots 0-15 s1p, 16-31 sup, then recycled for s4p/svp
            s1p = [sg.tile([P, 2, b_loc], FP8, tag="sg", name=f"s1_{c}")
                   for c in range(HT // 2)]
            ov_s1p = _Overlay()

            def mk_s1p(t):
                return lambda: sign_step(0, t, rtile("rs1", bld.md[0], t), s1p)

            for t in range(HT):
                ov_s1p.add(mk_s1p(t))

            bld.layer(1, bld.w2t, sx2, KT_X, mtp, "mt", wp, "w",
                      [P, KT_X, P], ov_s1p, 6)
            bld.issue_ar(1, (0, HT), 1)
            ov_s1p.drain()

            # ---------------- phase B: L3 (+sup overlay) ----------------
            sup = [sg.tile([P, 2, b_loc], FP8, tag="sg", name=f"su_{c}")
                   for c in range(HT // 2)]
            ov_sup = _Overlay()

            def mk_sup(t):
                return pair_parts(0, 1, t,
                                  lambda t=t: rtile("rm1", bld.md[0], t),
                                  lambda t=t: rtile("rm2", bld.md[1], t), sup)

            parts = [mk_sup(t) for t in range(HT)]
            for t in range(HT):
                ov_sup.add(parts[t][0])
                if t >= 1:
                    ov_sup.add(parts[t - 1][1])
            ov_sup.add(parts[HT - 1][1])

            if SPLIT_AR:
                l3_cb = lambda m: bld.issue_ar(2, (0, HT // 2), (2, 0)) if m == HT // 2 - 1 else None
            else:
                l3_cb = None
            bld.layer(2, bld.w3t, s1p, HT, mtp, "mt", wp, "w",
                      [P, HT, P], ov_sup, 6, ar_cb=l3_cb)
            if SPLIT_AR:
                bld.issue_ar(2, (HT // 2, HT), (2, 1))
            else:
                bld.issue_ar(2, (0, HT), 2)
            ov_sup.drain()

            # ---------------- phase C: L4 (+s4p first half overlay) ----------------
            s4p = [sg.tile([P, 2, b_loc], FP8, tag="sg", name=f"s4_{c}")
                   for c in range(HT // 2)]
            svp = [sg.tile([P, 2, b_loc], FP8, tag="sg", name=f"sv_{c}")
                   for c in range(HT // 2)]

            def mk_s4p(t):
                return lambda: sign_step(3, t, rtile("rs4", bld.md[3], t), s4p)

            ov_l4 = _Overlay()

            def l4_ar_cb(m):
                if SPLIT_AR and m == HT // 2 - 1:
                    bld.issue_ar(3, (0, HT // 2), (3, 0))
                    for t in range(HT // 2):
                        ov_l4.add(mk_s4p(t))

            bld.layer(3, bld.w4t, sup, HT, mtp, "mt", wp, "w",
                      [P, HT, P], ov_l4, 2, ar_cb=l4_ar_cb)
            if SPLIT_AR:
                bld.issue_ar(3, (HT // 2, HT), (3, 1))
            else:
                bld.issue_ar(3, (0, HT), 3)
            ov_l4.drain()

            # ---------------- head ----------------
            bsb = bsp.tile([P, 2 * D_OUT], F32, tag="bsb", name="bsb")
            nc.sync.dma_start(out=bsb[:, :], in_=bld.bias56[:, :])

            ov_head = _Overlay()
            for t in range(0 if not SPLIT_AR else HT // 2, HT):
                ov_head.add(mk_s4p(t))

            def mk_svp(t):
                return pair_parts(2, 3, t,
                                  lambda t=t: rtile("rm3", bld.md[2], t),
                                  lambda t=t: rtile("rm4", bld.md[3], t), svp)

            partsv = [mk_svp(t) for t in range(HT)]
            for t in range(HT):
                ov_head.add(partsv[t][0])
                if t >= 1:
                    ov_head.add(partsv[t - 1][1])
            ov_head.add(partsv[HT - 1][1])
            if not OVERLAP:
                ov_head.drain()

            y6 = [y6p.tile([P, D_OUT], F32, tag="y6", name=f"y6_{bt}")
                  for bt in range(bld.bt_tiles)]

            def wslice(li, kc, jb):
                ws = wp.tile([P, 2, 500], FP8, tag="w", name=f"ws{li}_{kc}_{jb}")
                nc.sync.dma_start(
                    out=ws[:, :, :],
                    in_=bld.w56[:, 2 * kc:2 * kc + 2, li * D_OUT + jb:li * D_OUT + jb + 500],
                )
                return ws

            def head_group(li, acts, bts, drain_per_kc, out_cb):
                pss = {}
                for bt in bts:
                    for jb in (0, 500):
                        pss[(bt, jb)] = bld.psum.tile([P, 500], F32, tag="ps",
                                                      name=f"ph{li}_{bt}_{jb}")
                for kc in range(HT // 2):
                    # drain BEFORE emitting kc's matmuls: overlay items include
                    # the sign producers of acts[kc] — emission order IS the
                    # dependency order in the tile framework
                    if OVERLAP and len(ov_head):
                        ov_head.drain(drain_per_kc)
                    # bt-inner with both jb slices live: consecutive matmuls per
                    # bt share the stationary acts tile (one LDWEIGHTS per pair)
                    wss = {jb: wslice(li, kc, jb) for jb in (0, 500)}
                    for bt in bts:
                        for jb in (0, 500):
                            nc.tensor.matmul(
                                pss[(bt, jb)][:, :],
                                lhsT=acts[kc][:, :, bt * P:(bt + 1) * P],
                                rhs=wss[jb][:, :, :],
                                start=(kc == 0), stop=(kc == HT // 2 - 1),
                                perf_mode=DR,
                            )
                for bt in bts:
                    for jb in (0, 500):
                        out_cb(bt, jb, pss[(bt, jb)])

            def out_b(bt, jb, ps):
                nc.scalar.activation(y6[bt][:, jb:jb + 500], ps[:, :], AF.Copy)

            head_group(1, s4p, [0, 1, 2, 3], 2, out_b)
            head_group(1, s4p, [4, 5, 6, 7], 2, out_b)

            def softmax_bt(bt, y):
                mx = bld.small.tile([P, 1], F32, tag="mx", name=f"mx{bt}")
                nc.vector.tensor_reduce(out=mx[:, :], in_=y[:, :], axis=AX.X, op=ALU.max)
                negmx = bld.small.tile([P, 1], F32, tag="negmx", name=f"nmx{bt}")
                nc.vector.tensor_scalar_mul(negmx[:, :], mx[:, :], -1.0)
                escr = yp.tile([P, 2 * D_OUT], F32, tag="escr", bufs=1, name=f"e{bt}")
                sume = bld.small.tile([P, 1], F32, tag="sume", name=f"se{bt}")
                nc.scalar.activation(escr[:, :], y[:, :], AF.Exp, bias=negmx[:, 0:1],
                                     scale=1.0, accum_out=sume[:, 0:1])
                lse = bld.small.tile([P, 1], F32, tag="lse", name=f"lse{bt}")
                nc.scalar.activation(lse[:, :], sume[:, :], AF.Ln)
                nc.vector.tensor_scalar(
                    out=escr[:, :], in0=y[:, :], scalar1=mx[:, 0:1], scalar2=lse[:, 0:1],
                    op0=ALU.subtract, op1=ALU.subtract,
                )
                nc.sync.dma_start(out=bld.outp[bt * P:(bt + 1) * P, :], in_=escr[:, :])

            ybufs = {}

            def out_a(bt, jb, ps):
                if bt not in ybufs:
                    ybufs[bt] = yp.tile([P, 2 * D_OUT], F32, tag="y", name=f"y{bt}")
                y = ybufs[bt]
                nc.vector.tensor_tensor(
                    out=y[:, jb:jb + 500], in0=ps[:, :],
                    in1=bsb[:, jb:jb + 500], op=ALU.add,
                )
                if jb == 500:
                    nc.vector.tensor_tensor(
                        out=y[:, D_OUT:2 * D_OUT], in0=y6[bt][:, :],
                        in1=bsb[:, D_OUT:2 * D_OUT], op=ALU.add,
                    )
                    softmax_bt(bt, y)

            head_group(0, svp, [0, 1, 2, 3], 3, out_a)
            ov_head.drain()
            head_group(0, svp, [4, 5, 6, 7], 0, out_a)
    if finalize:
        nc.finalize()
    return nc


# ---------------- host side ----------------

def _prep_shared(w1, w2, w3, w4, w5, w6, b5, b6, g1, bt1, g2, bt2, g3, bt3, g4, bt4):
    def wtile(w, kt):
        a = np.sign(w).astype(np.float32)
        mt = a.shape[0] // P
        a4 = a.reshape(mt, P, kt, P)           # [m, j, t, p]
        return np.ascontiguousarray(a4.transpose(0, 3, 2, 1)).astype(f8np)

    def wmov(w):
        a = np.sign(w).astype(np.float32).T    # [4096, 1000]
        a3 = a.reshape(HT, P, a.shape[1])      # [t, p, j]
        return np.ascontiguousarray(a3.transpose(1, 0, 2)).astype(f8np)

    w56_h = np.ascontiguousarray(np.concatenate([wmov(w5), wmov(w6)], axis=2))
    bias56_h = np.ascontiguousarray(
        np.broadcast_to(
            np.concatenate([b5, b6]).astype(np.float32)[None, :], (P, 2 * D_OUT)
        )
    )

    def gcol(v):
        return np.ascontiguousarray(v.astype(np.float32).reshape(HT, P).T)  # [128, 32]

    gb_h = np.ascontiguousarray(
        np.concatenate([gcol(v) for v in (g1, bt1, g2, bt2, g3, bt3, g4, bt4)], axis=1)
    )

    return {
        "w1t": wtile(w1, KT_X),
        "w2t": wtile(w2, KT_X),
        "w3t": wtile(w3, HT),
        "w4t": wtile(w4, HT),
        "w56": w56_h,
        "bias56": bias56_h,
        "gb": gb_h,
    }


def _prep_x(xs_core):
    """[b_loc, 2048] f32 -> [8, 128, 2, b_loc] fp8 of sign(x), DR tiled."""
    b_loc = xs_core.shape[0]
    s = np.sign(xs_core.astype(np.float32)).T          # [2048, b]
    s5 = s.reshape(2, 4, 2, P, b_loc)                  # [half, c, i, p, b]
    s5 = s5.transpose(0, 1, 3, 2, 4)                   # [half, c, p, i, b]
    return np.ascontiguousarray(s5.reshape(8, P, 2, b_loc)).astype(f8np)


_program_cache = {}


def kernel(x, w1, b1, w2, b2, w3, b3, w4, b4, w5, b5, w6, b6,
           g1, bt1, g2, bt2, g3, bt3, g4, bt4, _trace=False, _tmpdir=None):
    x = np.asarray(x, dtype=np.float32)
    b_loc = B // N_CORES

    shared = _prep_shared(
        np.asarray(w1), np.asarray(w2), np.asarray(w3), np.asarray(w4),
        np.asarray(w5), np.asarray(w6), np.asarray(b5), np.asarray(b6),
        np.asarray(g1), np.asarray(bt1), np.asarray(g2), np.asarray(bt2),
        np.asarray(g3), np.asarray(bt3), np.asarray(g4), np.asarray(bt4),
    )

    in_maps = []
    for c in range(N_CORES):
        in_maps.append({"xs": _prep_x(x[c * b_loc:(c + 1) * b_loc, :]), **shared})

    key = (b_loc, N_CORES)
    if key not in _program_cache:
        _program_cache[key] = build_program(b_loc, N_CORES)
    nc = _program_cache[key]

    res = run_bass_kernel_spmd(
        nc, in_maps, list(range(N_CORES)), trace=_trace, tmpdir=_tmpdir
    )
    out = np.concatenate([res.results[c]["out"] for c in range(N_CORES)], axis=0)
    if _trace:
        return out, res
    return out
